# revision 8
# baseline (speedup 1.0000x reference)
"""Trainium2 Bass kernel for nn_LocalTypicalityMILHead (retrieval_knn).

Strategy (8 NeuronCores, row-sharded [n,n] distance computation):
  - Host passes x.T column-rolled per core (bf16) so each core's 768-row
    block sits at local columns [0, 768) -> identical SPMD program with
    compile-time offsets.
  - Each core computes feats.T = relu(W1.T @ x.T + b1) (bf16 matmuls, f32
    PSUM accumulate, feats stored bf16), sq_j = colsum(feats.T^2) in f32,
    then its G = feats_blk @ feats.T block fused into
    d2 = sq_i + sq_j - 2G via a rank-1 (-0.5*sq_j) PSUM accumulate plus one
    scalar-engine pass (scale=-2, bias=sq_i fp32).
  - Per-row stats streamed out: sum(D), sum(d2), cnt(d2<eps^2), min offdiag
    d2, pre-clamp diag d2, sq rows, logits rows, and the feats block.
  - Host finishes: Silverman bandwidth h from global stats; diag corrected
    for the bf16 rank-1 sq_j (d2_ii_raw = ddiag - (bf16(sq_i)-sq_i), pure
    f32-roundoff scale); typicality typ_i = exp(-d2_ii/(2h^2))/cnt, exact
    in f32 because every off-diagonal Gaussian term underflows to 0
    (guarded via the min-offdiag stat, full numpy fallback otherwise);
    then the tiny topk / key-instance / soft-label stage.
"""

import sys

if "/opt/trn_rl_repo" not in sys.path:
    sys.path.insert(0, "/opt/trn_rl_repo")

import numpy as np
import ml_dtypes

BF16 = ml_dtypes.bfloat16

N, IN, HID, NCLS = 6144, 1024, 512, 2
EPS = 19.0
K = 10
NCORES = 8
P = 128
FD = 512  # free-dim tile width


def build_program(n=N, in_=IN, hid=HID, ncores=NCORES):
    import concourse.mybir as mybir
    import concourse.tile as tile
    from concourse import bacc
    from concourse.bass import ds, ts

    dt = mybir.dt
    f32 = dt.float32
    bf16 = dt.bfloat16
    AF = mybir.ActivationFunctionType
    ALU = mybir.AluOpType
    AX = mybir.AxisListType

    B = n // ncores       # rows per core
    MI = B // P           # 128-row tiles per core
    NT = n // FD          # column tiles
    KF = in_ // P         # K chunks for the feature matmul
    KG = hid // P         # K chunks for the Gram matmul

    nc = bacc.Bacc("TRN2", target_bir_lowering=False, debug=False,
                   num_devices=ncores)

    xT = nc.dram_tensor("xT", [in_, n], bf16, kind="ExternalInput").ap()
    W1 = nc.dram_tensor("W1", [in_, hid], bf16, kind="ExternalInput").ap()
    b1 = nc.dram_tensor("b1", [hid], f32, kind="ExternalInput").ap()
    Wc = nc.dram_tensor("Wc", [hid, NCLS], bf16, kind="ExternalInput").ap()
    bc = nc.dram_tensor("bc", [NCLS], bf16, kind="ExternalInput").ap()

    rsD = nc.dram_tensor("rsD", [P, MI], f32, kind="ExternalOutput").ap()
    rsd2 = nc.dram_tensor("rsd2", [P, MI], f32, kind="ExternalOutput").ap()
    cnt = nc.dram_tensor("cnt", [P, MI], f32, kind="ExternalOutput").ap()
    mino = nc.dram_tensor("minoff", [P, MI], f32, kind="ExternalOutput").ap()
    ddia = nc.dram_tensor("ddiag", [P, MI], f32, kind="ExternalOutput").ap()
    sqb = nc.dram_tensor("sqb", [B], f32, kind="ExternalOutput").ap()
    lgt = nc.dram_tensor("logits", [MI, P, NCLS], f32, kind="ExternalOutput").ap()
    ftTb = nc.dram_tensor("ftTb", [KG, P, B], bf16, kind="ExternalOutput").ap()

    with tile.TileContext(nc) as tc:
        with (
            tc.tile_pool(name="persist", bufs=1) as persist,
            tc.tile_pool(name="xin", bufs=3) as xin,
            tc.tile_pool(name="work", bufs=4) as work,
            tc.tile_pool(name="scr", bufs=3) as scr,
            tc.tile_pool(name="stats", bufs=2) as statsp,
            tc.tile_pool(name="dram", bufs=1, space="DRAM") as dramp,
            tc.tile_pool(name="psum", bufs=4, space="PSUM") as psum,
            tc.tile_pool(name="psum2", bufs=1, space="PSUM") as psum2,
        ):
            ft = [persist.tile([P, n], bf16, tag=f"ft{m}", name=f"ft{m}")
                  for m in range(KG)]
            sqv = persist.tile([1, n], f32, tag="sqv")
            sqvb = persist.tile([1, n], bf16, tag="sqvb")
            sqrb = persist.tile([P, MI], f32, tag="sqrb")
            onesPb = persist.tile([1, P], bf16, tag="onesPb")
            neghb = persist.tile([1, P], bf16, tag="neghb")
            onesC = persist.tile([P, 1], f32, tag="onesC")
            w1sb = persist.tile([P, KF, hid], bf16, tag="w1sb")
            wcsb = persist.tile([P, KG, NCLS], bf16, tag="wcsb")
            bcsb = persist.tile([1, NCLS], bf16, tag="bcsb")
            b1sb = persist.tile([P, KG], f32, tag="b1sb")
            outD = persist.tile([P, MI], f32, tag="outD")
            outd2 = persist.tile([P, MI], f32, tag="outd2")
            outc = persist.tile([P, MI], f32, tag="outc")
            outm = persist.tile([P, MI], f32, tag="outm")
            outdd = persist.tile([P, MI], f32, tag="outdd")

            nc.any.memset(onesPb, 1.0)
            nc.any.memset(neghb, -0.5)
            nc.any.memset(onesC, 1.0)
            nc.sync.dma_start(w1sb, W1.rearrange("(kc p) m -> p kc m", p=P))
            nc.sync.dma_start(wcsb, Wc.rearrange("(kc p) m -> p kc m", p=P))
            nc.sync.dma_start(bcsb, bc[None, :])
            nc.sync.dma_start(b1sb, b1.rearrange("(mc p) -> p mc", p=P))

            xTr = xT.rearrange("(kc p) j -> p kc j", p=P)

            # ---- feats.T = relu(W1.T @ x.T + b1), by column tile ----
            for ni in range(NT):
                nsl = ds(ni * FD, FD)
                xt = xin.tile([P, KF, FD], bf16, tag="xt")
                nc.sync.dma_start(xt, xTr[:, :, nsl])
                for m in range(KG):
                    pt = psum.tile([P, FD], f32, tag="pt")
                    for k in range(KF):
                        nc.tensor.matmul(
                            pt,
                            w1sb[:, k, ts(m, P)],
                            xt[:, k, :],
                            start=(k == 0),
                            stop=(k == KF - 1),
                        )
                    nc.scalar.activation(
                        ft[m][:, nsl], pt, AF.Relu,
                        bias=b1sb[:, m : m + 1], scale=1.0,
                    )
                # sq_j = sum_h feats[j,h]^2 for this column tile (f32)
                acc = scr.tile([P, FD], f32, tag="sqacc")
                tmp = scr.tile([P, FD], f32, tag="sqtmp")
                nc.vector.tensor_tensor(acc, ft[0][:, nsl], ft[0][:, nsl], ALU.mult)
                for m in range(1, KG):
                    nc.vector.tensor_tensor(tmp, ft[m][:, nsl], ft[m][:, nsl], ALU.mult)
                    nc.vector.tensor_tensor(acc, acc, tmp, ALU.add)
                pq = psum2.tile([1, FD], f32, tag="pq")
                nc.tensor.matmul(pq, onesC, acc, start=True, stop=True)
                nc.scalar.copy(sqv[:, nsl], pq)
                nc.scalar.copy(sqvb[:, nsl], pq)

            # per-partition sq for this core's own block rows:
            # sqrb[p, mi] = sq[mi*P + p], via a DRAM bounce (SBUF->SBUF
            # partition scatter is not a legal single DMA). Also ship sq.
            sqd = dramp.tile([B], f32, tag="sqd")
            nc.sync.dma_start(sqd, sqv[0:1, 0:B])
            nc.sync.dma_start(sqrb, sqd.rearrange("(mi p) -> p mi", p=P))
            nc.sync.dma_start(sqb, sqv[0:1, 0:B])

            # ---- Gram block + distance stats ----
            for mi in range(MI):
                msl = ds(mi * P, P)
                nd = (mi * P) // FD
                off = mi * P - nd * FD
                accD = statsp.tile([P, NT], f32, tag="accD")
                accd2 = statsp.tile([P, NT], f32, tag="accd2")
                accc = statsp.tile([P, NT], f32, tag="accc")
                accm = statsp.tile([P, NT], f32, tag="accm")
                for ni in range(NT):
                    nsl = ds(ni * FD, FD)
                    pt = psum.tile([P, FD], f32, tag="pt")
                    for k in range(KG):
                        nc.tensor.matmul(
                            pt,
                            ft[k][:, msl],
                            ft[k][:, nsl],
                            start=(k == 0),
                            stop=False,
                        )
                    nc.tensor.matmul(pt, neghb, sqvb[:, nsl],
                                     start=False, stop=True)
                    # t = -2*(G - 0.5*sq_j) + sq_i = sq_i + sq_j - 2G
                    t = work.tile([P, FD], f32, tag="t")
                    nc.scalar.activation(
                        t, pt, AF.Identity, bias=sqrb[:, mi : mi + 1], scale=-2.0
                    )
                    if ni == nd:
                        # pre-clamp diagonal extraction + diag-masked min
                        msel = scr.tile([P, FD], f32, tag="dscr")
                        nc.gpsimd.affine_select(
                            msel, t, pattern=[[1, FD]],
                            compare_op=ALU.not_equal, fill=1e30,
                            base=-off, channel_multiplier=-1,
                        )
                        nc.vector.tensor_reduce(
                            accm[:, ni : ni + 1], msel, axis=AX.X, op=ALU.min
                        )
                        dsel = scr.tile([P, FD], f32, tag="cscr")
                        nc.gpsimd.affine_select(
                            dsel, t, pattern=[[1, FD]],
                            compare_op=ALU.is_equal, fill=0.0,
                            base=-off, channel_multiplier=-1,
                        )
                        nc.vector.tensor_reduce(
                            outdd[:, mi : mi + 1], dsel, axis=AX.X, op=ALU.add
                        )
                    else:
                        nc.vector.tensor_reduce(
                            accm[:, ni : ni + 1], t, axis=AX.X, op=ALU.min
                        )
                    # clamp -> d2c; accumulate sum(d2c)
                    nc.vector.tensor_scalar(
                        t, t, 1e-12, None, ALU.max, op1=ALU.add,
                        accum_out=accd2[:, ni : ni + 1],
                    )
                    # D = sqrt(d2c); accumulate sum(D)
                    dscr = scr.tile([P, FD], f32, tag="dscr")
                    nc.scalar.activation(
                        dscr, t, AF.Sqrt, accum_out=accD[:, ni : ni + 1]
                    )
                    # neighbor count: d2c < EPS^2
                    cscr = scr.tile([P, FD], f32, tag="cscr")
                    nc.vector.tensor_scalar(
                        cscr, t, EPS * EPS, None, ALU.is_lt, op1=ALU.add,
                        accum_out=accc[:, ni : ni + 1],
                    )
                nc.vector.tensor_reduce(outD[:, mi : mi + 1], accD, axis=AX.X, op=ALU.add)
                nc.vector.tensor_reduce(outd2[:, mi : mi + 1], accd2, axis=AX.X, op=ALU.add)
                nc.vector.tensor_reduce(outc[:, mi : mi + 1], accc, axis=AX.X, op=ALU.add)
                nc.vector.tensor_reduce(outm[:, mi : mi + 1], accm, axis=AX.X, op=ALU.min)

                # logits rows for this tile
                pl = psum2.tile([P, NCLS], f32, tag="pl")
                for k in range(KG):
                    nc.tensor.matmul(
                        pl, ft[k][:, msl], wcsb[:, k, :],
                        start=(k == 0), stop=False,
                    )
                nc.tensor.matmul(pl, onesPb, bcsb, start=False, stop=True)
                lg = scr.tile([P, NCLS], f32, tag="lg")
                nc.scalar.copy(lg, pl)
                nc.sync.dma_start(lgt[mi], lg)

            nc.sync.dma_start(rsD, outD)
            nc.sync.dma_start(rsd2, outd2)
            nc.sync.dma_start(cnt, outc)
            nc.sync.dma_start(mino, outm)
            nc.sync.dma_start(ddia, outdd)
            for m in range(KG):
                nc.sync.dma_start(ftTb[m], ft[m][:, 0:B])

    nc.compile()
    return nc


_NC_CACHE = {}


def _get_program(n=N, in_=IN, hid=HID, ncores=NCORES):
    key = (n, in_, hid, ncores)
    if key not in _NC_CACHE:
        _NC_CACHE[key] = build_program(n, in_, hid, ncores)
    return _NC_CACHE[key]


def make_in_maps(x, W1, b1, Wc, bc, ncores=NCORES):
    n = x.shape[0]
    B = n // ncores
    xT = np.ascontiguousarray(np.asarray(x, np.float32).T.astype(BF16))
    W1 = np.ascontiguousarray(np.asarray(W1, np.float32).astype(BF16))
    b1 = np.ascontiguousarray(np.asarray(b1, np.float32))
    Wc = np.ascontiguousarray(np.asarray(Wc, np.float32).astype(BF16))
    bc = np.ascontiguousarray(np.asarray(bc, np.float32).astype(BF16))
    maps = []
    for c in range(ncores):
        xTc = np.ascontiguousarray(np.roll(xT, -c * B, axis=1))
        maps.append({"xT": xTc, "W1": W1, "b1": b1, "Wc": Wc, "bc": bc})
    return maps


def gather_results(results, n, hid, ncores=NCORES):
    """Unshard per-core outputs into full-length arrays."""
    B = n // ncores
    feats = np.empty((n, hid), np.float32)
    logits = np.empty((n, NCLS), np.float32)
    out = {k: np.empty(n, np.float64) for k in
           ("rsD", "rsd2", "cnt", "minoff", "ddiag", "sqb")}
    for c in range(ncores):
        r = results[c]
        sl = slice(c * B, (c + 1) * B)
        for k in out:
            if k == "sqb":
                out[k][sl] = np.asarray(r[k]).reshape(-1)
            else:
                # [P, MI] -> row index mi*P + p
                out[k][sl] = np.ascontiguousarray(r[k].T).reshape(B)
        logits[sl] = r["logits"].reshape(B, NCLS)
        feats[sl] = r["ftTb"].reshape(hid, B).T.astype(np.float32)
    return feats, logits, out


def host_finish(feats, logits, st, n, bag_label):
    """Bandwidth + typicality + key-instance selection + soft labels."""
    f64 = np.float64
    S_D = f64(st["rsD"].sum())
    S_d2 = f64(st["rsd2"].sum())
    n2 = f64(n) * f64(n)
    mean = S_D / n2
    var = (S_d2 - n2 * mean * mean) / (n2 - 1.0)
    s = np.float32(np.sqrt(max(var, 0.0)))
    mean32 = np.float32(mean)
    if not np.isfinite(s) or s < 1e-6:
        s = np.float32(mean32 + 1e-6)
    h = np.float32(max(1.06 * s * n ** (-0.2), 1e-3))
    inv2h2 = np.float32(1.0 / (2.0 * h * h))

    minoff_g = st["minoff"].min()
    guard_ok = np.isfinite(minoff_g) and (minoff_g * f64(inv2h2) > 50.0)

    cntv = st["cnt"].astype(np.float32)
    if guard_ok:
        # correct the diagonal for the bf16 rank-1 sq_j contribution
        sqf = st["sqb"].astype(np.float32)
        sq_bf = sqf.astype(BF16).astype(np.float32)
        draw = st["ddiag"].astype(np.float32) - (sq_bf - sqf)
        ddiag = np.maximum(draw, 1e-12).astype(np.float32)
        ker_diag = np.exp(-ddiag * inv2h2) * (ddiag < EPS * EPS)
        typ = np.where(cntv > 0, ker_diag / np.maximum(cntv, 1.0), 0.0)
        typ = typ.astype(np.float32)
    else:
        # exact but slow fallback: full distance matrix on host
        sq = np.sum(feats * feats, axis=1)
        G = feats @ feats.T
        d2 = np.maximum(sq[:, None] + sq[None, :] - 2.0 * G, 1e-12)
        D = np.sqrt(d2).astype(np.float32)
        s = D.std(ddof=1, dtype=np.float64)
        m_ = D.mean(dtype=np.float64)
        s = np.float32(s if s >= 1e-6 else m_ + 1e-6)
        h = np.float32(max(1.06 * s * n ** (-0.2), 1e-3))
        ker = np.exp(-d2 / (2.0 * h * h))
        nbr = D < EPS
        cntv = nbr.sum(1).astype(np.float32)
        typ = np.where(
            cntv > 0, (ker * nbr).sum(1) / np.maximum(cntv, 1.0), 0.0
        ).astype(np.float32)

    # ---- key instance selection (ties broken like jax.lax.top_k) ----
    bag = int(bag_label)
    pos_score = typ if bag == 1 else -typ
    pos_idx = np.argsort(-pos_score, kind="stable")[:K]
    pos_mask = np.zeros(n, bool)
    pos_mask[pos_idx] = True

    sqh = np.sum(feats * feats, axis=1)

    def dcols(idx):
        G = feats @ feats[idx].T
        d2 = sqh[:, None] + sqh[idx][None, :] - 2.0 * G
        return np.sqrt(np.maximum(d2, 1e-12)).astype(np.float32)

    Dp = dcols(pos_idx)                      # [n, K] == D[:, pos_idx]
    rows = np.where(pos_mask[:, None], -1.0, Dp).T  # [K, n] == masked D[pos_idx]
    far_idx = np.argsort(-rows, axis=1, kind="stable")[:, :K]
    cand = np.zeros(n, bool)
    cand[far_idx.reshape(-1)] = True
    cand &= ~pos_mask
    neg_base = -typ if bag == 1 else typ
    nb = np.where(cand, neg_base, -np.inf)
    neg_idx = np.argsort(-nb, kind="stable")[:K]
    Dn = dcols(neg_idx)

    mp, mn = Dp < EPS, Dn < EPS
    pw = (typ[pos_idx] * mp).sum(1, dtype=np.float32)
    nw = (typ[neg_idx] * mn).sum(1, dtype=np.float32)
    tot = pw + nw
    use_w = ((mp.sum(1) + mn.sum(1)) > 0) & (tot > 0)
    p = pw / np.where(tot > 0, tot, 1.0)
    soft_w = np.stack([1.0 - p, p], axis=1)
    ps = np.exp(-Dp).mean(1)
    ns = np.exp(-Dn).mean(1)
    t2 = ps + ns + 1e-6
    soft_f = np.stack([ns / t2, ps / t2], axis=1)
    soft = np.where(use_w[:, None], soft_w, soft_f).astype(np.float32)

    return logits, soft, typ


def kernel(x, W1, b1, Wc, bc, bag_label=1, **_ignored):
    from concourse.bass_utils import run_bass_kernel_spmd

    x = np.asarray(x)
    n, in_ = x.shape
    hid = W1.shape[1]
    nc = _get_program(n, in_, hid, NCORES)
    in_maps = make_in_maps(x, W1, b1, Wc, bc, NCORES)
    res = run_bass_kernel_spmd(nc, in_maps, list(range(NCORES))).results
    feats, logits, st = gather_results(res, n, hid, NCORES)
    try:
        bag = int(np.asarray(bag_label).item())
    except Exception:
        bag = int(bag_label)
    return host_finish(feats, logits, st, n, bag)
